# revision 1
# baseline (speedup 1.0000x reference)
"""H2GCN forward on 8 TRN2 NeuronCores.

Strategy (dest-row sharding, per spec hint):
  - Nodes (rows of x / segment dim) sharded 8 ways; edges partitioned by
    destination row; 256x256 linears replicated.
  - Normalized adjacency D^-1/2 A D^-1/2 is separable: scale sources once
    (x~ = dis * x, done on host as input prep), SpMM is then a pure 0/1
    gather + segment-sum; per-hop output rows rescaled by dis / dis^2 on
    device.
  - SpMM on device: dma_gather (SWDGE indirect DMA) fetches 1KB source rows
    from HBM at near line rate; segment-sum runs on TensorE as
    S_chunk.T @ msg_chunk where S_chunk is a 0/1 selection matrix built on
    VectorE via is_equal(dest_local, iota).
  - Between hops: AllGather of the per-core [6250,256] hop-1 result.
  - GEMMs: hop outputs transposed on TensorE (feats -> partitions), linears
    as W.T @ curT in bf16 (fp32 accumulate), relu+bias fused on ScalarE,
    classifier contracts the 768-dim concat, final transpose back.
"""

import os
import sys

import numpy as np

sys.path.insert(0, "/opt/trn_rl_repo")

import ml_dtypes  # noqa: E402

import concourse.bass as bass  # noqa: E402
import concourse.tile as tile  # noqa: E402
from concourse import bacc, bass_utils, mybir  # noqa: E402

N = 50000  # nodes
D = 256  # in/hidden channels
CO = 64  # out channels
NCORES = 8
R = N // NCORES  # 6250 dest rows per core
PB = 128  # dest block size (PSUM partition dim)
NBLK = (R + PB - 1) // PB  # 49 dest blocks per core
SPLIT = 32768  # int16 index limit for dma_gather
GRP = 2  # dest blocks per gather group
ROWG = 512  # GEMM row-group size

f32 = mybir.dt.float32
f32r = mybir.dt.float32r
bf16 = mybir.dt.bfloat16
i16 = mybir.dt.int16

_prog_cache = {}


def _preprocess(x, edge_index):
    """Host-side graph prep. Returns per-core tensors + shared layout."""
    row = edge_index[0].astype(np.int64)
    col = edge_index[1].astype(np.int64)
    loops = np.arange(N, dtype=np.int64)
    er = np.concatenate([row, loops])
    ec = np.concatenate([col, loops])
    deg = np.bincount(er, minlength=N).astype(np.float32)
    dis = np.where(deg > 0, deg ** -0.5, 0.0).astype(np.float32)

    order = np.argsort(er, kind="stable")
    er = er[order]
    ec = ec[order]

    xt = (x * dis[:, None]).astype(np.float32)  # gather source for hop 1

    # per (core, block): lo/hi source lists
    lo_lists = [[None] * NBLK for _ in range(NCORES)]
    hi_lists = [[None] * NBLK for _ in range(NCORES)]
    dl_lists_lo = [[None] * NBLK for _ in range(NCORES)]
    dl_lists_hi = [[None] * NBLK for _ in range(NCORES)]
    for c in range(NCORES):
        base = c * R
        for b in range(NBLK):
            d0 = base + b * PB
            d1 = min(base + (b + 1) * PB, base + R)
            e0 = np.searchsorted(er, d0, side="left")
            e1 = np.searchsorted(er, d1, side="left")
            srcs = ec[e0:e1]
            dl = (er[e0:e1] - d0).astype(np.float32)
            m = srcs < SPLIT
            lo_lists[c][b] = srcs[m].astype(np.int16)
            dl_lists_lo[c][b] = dl[m]
            hi_lists[c][b] = (srcs[~m] - SPLIT).astype(np.int16)
            dl_lists_hi[c][b] = dl[~m]

    # shared chunk counts per block position (max over cores)
    CLO = [0] * NBLK
    CHI = [0] * NBLK
    for b in range(NBLK):
        CLO[b] = max((len(lo_lists[c][b]) + PB - 1) // PB for c in range(NCORES))
        CHI[b] = max((len(hi_lists[c][b]) + PB - 1) // PB for c in range(NCORES))

    # layout: groups of GRP blocks; per group: [b0_lo|b1_lo| ... |b0_hi|b1_hi]
    ngroups = (NBLK + GRP - 1) // GRP
    groups = []  # (blocks, lo_off_ch, lo_nch, hi_off_ch, hi_nch, blk_chunks)
    totch = 0
    for g in range(ngroups):
        blocks = list(range(g * GRP, min((g + 1) * GRP, NBLK)))
        lo_off = totch
        lo_nch = sum(CLO[b] for b in blocks)
        hi_off = lo_off + lo_nch
        hi_nch = sum(CHI[b] for b in blocks)
        # chunk spans (within group tile) per block: (lo_start, nlo, hi_start, nhi)
        blk_chunks = {}
        o = 0
        for b in blocks:
            blk_chunks[b] = (o, CLO[b])
            o += CLO[b]
        for b in blocks:
            blk_chunks[b] = blk_chunks[b] + (o, CHI[b])
            o += CHI[b]
        groups.append((blocks, lo_off, lo_nch, hi_off, hi_nch, blk_chunks))
        totch += lo_nch + hi_nch

    tot_slots = totch * PB

    # per-core slot arrays
    idx_tiles = []
    dl_tiles = []
    for c in range(NCORES):
        idxv = np.zeros(tot_slots, dtype=np.int16)
        dlv = np.full(tot_slots, 300.0, dtype=np.float32)
        for blocks, lo_off, lo_nch, hi_off, hi_nch, _ in groups:
            o = lo_off * PB
            for b in blocks:
                s = lo_lists[c][b]
                idxv[o : o + len(s)] = s
                dlv[o : o + len(s)] = dl_lists_lo[c][b]
                o += CLO[b] * PB
            o = hi_off * PB
            for b in blocks:
                s = hi_lists[c][b]
                idxv[o : o + len(s)] = s
                dlv[o : o + len(s)] = dl_lists_hi[c][b]
                o += CHI[b] * PB
        # idx tile [128, tot_slots/16]: idx i at (i%16, i//16), replicated x8
        it = idxv.reshape(-1, 16).T  # [16, S/16]
        idx_tiles.append(np.tile(it, (8, 1)).copy())
        dl_tiles.append(dlv.reshape(-1, PB).T.copy())  # [128, totch]

    dis_t = []
    dis2_t = []
    for c in range(NCORES):
        dv = np.zeros((PB, NBLK), dtype=np.float32)
        for b in range(NBLK):
            d0 = c * R + b * PB
            n = min(PB, c * R + R - d0)
            dv[:n, b] = dis[d0 : d0 + n]
        dis_t.append(dv)
        dis2_t.append(dv * dv)

    layout = (tuple(CLO), tuple(CHI))
    return xt, idx_tiles, dl_tiles, dis_t, dis2_t, groups, totch, layout


def _build_program(groups, totch):
    """Build the (core-shared) Bass program."""
    nc = bacc.Bacc("TRN2", target_bir_lowering=False, debug=False,
                   num_devices=NCORES)

    def din(name, shape, dt):
        return nc.dram_tensor(name, list(shape), dt, kind="ExternalInput")

    xt_d = din("xt", (N, D), bf16)
    xTr_d = din("xTr", (D, R), bf16)
    idx_d = din("idx", (128, totch * PB // 16), i16)
    dl_d = din("dl", (128, totch), f32)
    dis_d = din("dis", (PB, NBLK), f32)
    dis2_d = din("dis2", (PB, NBLK), f32)
    w0_d = din("w0", (D, D), bf16)
    w1_d = din("w1", (D, D), bf16)
    w2_d = din("w2", (D, D), bf16)
    wc_d = din("wc", (3 * D, CO), bf16)
    biash_d = din("biash", (128, 6), f32)  # [:, k*2+fo] = b_k[fo*128:...]
    bc_d = din("bc", (CO, 1), f32)
    iota_d = din("iota", (128, 128), f32)
    idb_d = din("idb", (128, 128), bf16)
    idf_d = din("idf", (128, 128), f32)
    out_d = nc.dram_tensor("out", [R, CO], f32, kind="ExternalOutput")

    nrowg = (R + ROWG - 1) // ROWG

    with tile.TileContext(nc) as tc:
        with (
            tc.tile_pool(name="const", bufs=1) as constp,
            tc.tile_pool(name="msg", bufs=2) as msgp,
            tc.tile_pool(name="sel", bufs=4) as selp,
            tc.tile_pool(name="scal", bufs=3) as scalp,
            tc.tile_pool(name="curT", bufs=1) as curtp,
            tc.tile_pool(name="hT", bufs=1) as htp,
            tc.tile_pool(name="xts", bufs=2) as xtsp,
            tc.tile_pool(name="yt", bufs=2) as ytp,
            tc.tile_pool(name="spsum", bufs=2, space="PSUM") as spsump,
            tc.tile_pool(name="tpsum", bufs=2, space="PSUM") as tpsump,
            tc.tile_pool(name="gpsum", bufs=2, space="PSUM") as gpsump,
            tc.tile_pool(name="ypsum", bufs=1, space="PSUM") as ypsump,
            tc.tile_pool(name="dram", bufs=1, space="DRAM") as dramp,
        ):
            # ---- constants to SBUF ----
            idx_t = constp.tile([128, totch * PB // 16], i16)
            nc.sync.dma_start(out=idx_t[:], in_=idx_d[:, :])
            dl_t = constp.tile([128, totch], f32)
            nc.sync.dma_start(out=dl_t[:], in_=dl_d[:, :])
            dis_t = constp.tile([PB, NBLK], f32)
            nc.sync.dma_start(out=dis_t[:], in_=dis_d[:, :])
            dis2_t = constp.tile([PB, NBLK], f32)
            nc.sync.dma_start(out=dis2_t[:], in_=dis2_d[:, :])
            iota3 = constp.tile([128, 1, 128], f32)
            nc.sync.dma_start(out=iota3[:, 0, :], in_=iota_d[:, :])
            idb_t = constp.tile([128, 128], bf16)
            nc.sync.dma_start(out=idb_t[:], in_=idb_d[:, :])
            idf_t = constp.tile([128, 128], f32)
            nc.sync.dma_start(out=idf_t[:], in_=idf_d[:, :])
            w0_t = constp.tile([128, 2 * D], bf16)  # [:, h*D:(h+1)*D] = W0[h*128:...]
            nc.sync.dma_start(out=w0_t[:, 0:D], in_=w0_d[0:128, :])
            nc.sync.dma_start(out=w0_t[:, D : 2 * D], in_=w0_d[128:256, :])
            w1_t = constp.tile([128, 2 * D], bf16)
            nc.sync.dma_start(out=w1_t[:, 0:D], in_=w1_d[0:128, :])
            nc.sync.dma_start(out=w1_t[:, D : 2 * D], in_=w1_d[128:256, :])
            w2_t = constp.tile([128, 2 * D], bf16)
            nc.sync.dma_start(out=w2_t[:, 0:D], in_=w2_d[0:128, :])
            nc.sync.dma_start(out=w2_t[:, D : 2 * D], in_=w2_d[128:256, :])
            wc_t = constp.tile([128, 6 * CO], bf16)
            for s in range(6):
                nc.sync.dma_start(out=wc_t[:, s * CO : (s + 1) * CO],
                                  in_=wc_d[s * 128 : (s + 1) * 128, :])
            biash_t = constp.tile([128, 6], f32)
            nc.sync.dma_start(out=biash_t[:], in_=biash_d[:, :])
            bc_t = constp.tile([CO, 1], f32)
            nc.sync.dma_start(out=bc_t[:], in_=bc_d[:, :])

            # persistent transposed activations
            curT = [curtp.tile([128, NBLK * PB], bf16, tag=f"curT{h}", name=f"curT{h}")
                    for h in range(2)]  # [fi half][feat, row]
            hT = [htp.tile([128, R], bf16, tag=f"hT{k}{fo}", name=f"hT{k}{fo}")
                  for k in range(3) for fo in range(2)]

            def hT_at(k, fo):
                return hT[k * 2 + fo]

            xt2_local = dramp.tile([R, D], bf16)
            xt2_full = dramp.tile([N, D], bf16)

            # ---------------- SpMM hop ----------------
            def hop(h, src_ap_full, src_ap_hi, cur_half_a, cur_half_b):
                for blocks, lo_off, lo_nch, hi_off, hi_nch, blk_chunks in groups:
                    g_nch = lo_nch + hi_nch
                    g_off = lo_off  # global chunk offset of this group
                    msg = msgp.tile([128, g_nch, D], bf16, tag="msg")
                    MAXCH = 8  # >1024 idxs per dma_gather faults the device
                    for src_ap, nch, ch0, off in (
                        (src_ap_full, lo_nch, 0, lo_off),
                        (src_ap_hi, hi_nch, lo_nch, hi_off),
                    ):
                        for p0 in range(0, nch, MAXCH):
                            pn = min(MAXCH, nch - p0)
                            nidx = pn * PB
                            nc.gpsimd.dma_gather(
                                msg[:, ch0 + p0 : ch0 + p0 + pn, :],
                                src_ap,
                                idx_t[:, (off + p0) * PB // 16
                                      : (off + p0 + pn) * PB // 16],
                                nidx, nidx, D,
                            )
                    for b in blocks:
                        lo_s, nlo, hi_s, nhi = blk_chunks[b]
                        nch_b = nlo + nhi
                        ps = spsump.tile([128, D], f32, tag="sp")
                        S = selp.tile([128, nch_b, 128], bf16, tag="S")
                        for s0, ns, gch in ((0, nlo, g_off + lo_s),
                                            (nlo, nhi, g_off + hi_s)):
                            if ns:
                                nc.vector.tensor_tensor(
                                    out=S[:, s0 : s0 + ns, :],
                                    in0=dl_t[:, gch : gch + ns]
                                        .to_broadcast([128, ns, 128]),
                                    in1=iota3[:, :, :].to_broadcast([128, ns, 128]),
                                    op=mybir.AluOpType.is_equal,
                                )
                        chunks = list(range(lo_s, lo_s + nlo)) + \
                            list(range(hi_s, hi_s + nhi))
                        for j, ch in enumerate(chunks):
                            nc.tensor.matmul(
                                ps[:],
                                lhsT=S[:, j, :],
                                rhs=msg[:, ch, :],
                                start=(j == 0),
                                stop=(j == len(chunks) - 1),
                            )
                        nrow = min(PB, R - b * PB)
                        if h == 0:
                            s1 = scalp.tile([128, D], bf16, tag="s1")
                            nc.vector.tensor_scalar_mul(
                                s1[:], ps[:], dis2_t[:, b : b + 1])
                            nc.sync.dma_start(
                                out=xt2_local[b * PB : b * PB + nrow, :],
                                in_=s1[:nrow, :])
                        cur = scalp.tile([128, D], bf16, tag="cur")
                        nc.vector.tensor_scalar_mul(
                            cur[:], ps[:], dis_t[:, b : b + 1])
                        for half, ct in ((0, cur_half_a), (1, cur_half_b)):
                            tp = tpsump.tile([128, 128], bf16, tag="tp")
                            nc.tensor.transpose(
                                tp[:], cur[:, half * 128 : (half + 1) * 128],
                                idb_t[:])
                            nc.vector.tensor_copy(
                                out=ct[:, b * PB : (b + 1) * PB], in_=tp[:])

            # hop 1: gather from host-prescaled x~
            hop(0, xt_d[:, :], xt_d[SPLIT:N, :], curT[0], curT[1])

            # GEMM0 (x @ W0) from host-transposed xTr, f32r full precision
            for rg in range(nrowg):
                r0 = rg * ROWG
                nr = min(ROWG, R - r0)
                xa = xtsp.tile([128, ROWG], bf16, tag="xa")
                xb = xtsp.tile([128, ROWG], bf16, tag="xb")
                nc.sync.dma_start(out=xa[:, :nr], in_=xTr_d[0:128, r0 : r0 + nr])
                nc.sync.dma_start(out=xb[:, :nr], in_=xTr_d[128:256, r0 : r0 + nr])
                for fo in range(2):
                    gp = gpsump.tile([128, ROWG], f32, tag="gp")
                    nc.tensor.matmul(
                        gp[:, :nr],
                        lhsT=w0_t[:, fo * 128 + 0 : fo * 128 + 128],
                        rhs=xa[:, :nr], start=True, stop=False)
                    nc.tensor.matmul(
                        gp[:, :nr],
                        lhsT=w0_t[:, D + fo * 128 : D + fo * 128 + 128],
                        rhs=xb[:, :nr], start=False, stop=True)
                    nc.scalar.activation(
                        out=hT_at(0, fo)[:, r0 : r0 + nr], in_=gp[:, :nr],
                        func=mybir.ActivationFunctionType.Relu,
                        bias=biash_t[:, fo : fo + 1], scale=1.0)

            # AllGather hop-1 scaled output
            nc.gpsimd.collective_compute(
                "AllGather",
                mybir.AluOpType.bypass,
                replica_groups=[list(range(NCORES))],
                ins=[xt2_local[:].opt()],
                outs=[xt2_full[:].opt()],
            )

            # GEMM1 (cur1 @ W1) in bf16
            def gemm_bf(k, w_t, curA, curB):
                for rg in range(nrowg):
                    r0 = rg * ROWG
                    nr = min(ROWG, R - r0)
                    for fo in range(2):
                        gp = gpsump.tile([128, ROWG], f32, tag="gp")
                        nc.tensor.matmul(
                            gp[:, :nr],
                            lhsT=w_t[:, fo * 128 : fo * 128 + 128],
                            rhs=curA[:, r0 : r0 + nr], start=True, stop=False)
                        nc.tensor.matmul(
                            gp[:, :nr],
                            lhsT=w_t[:, D + fo * 128 : D + fo * 128 + 128],
                            rhs=curB[:, r0 : r0 + nr], start=False, stop=True)
                        nc.scalar.activation(
                            out=hT_at(k, fo)[:, r0 : r0 + nr], in_=gp[:, :nr],
                            func=mybir.ActivationFunctionType.Relu,
                            bias=biash_t[:, k * 2 + fo : k * 2 + fo + 1],
                            scale=1.0)

            gemm_bf(1, w1_t, curT[0], curT[1])

            # hop 2: gather from all-gathered xt2
            cur2T = [curtp.tile([128, NBLK * PB], bf16, tag=f"cur2T{h}", name=f"cur2T{h}")
                     for h in range(2)]
            hop(1, xt2_full[:, :], xt2_full[SPLIT:N, :], cur2T[0], cur2T[1])

            gemm_bf(2, w2_t, cur2T[0], cur2T[1])

            # classifier y = relu-concat @ Wc + bc, computed transposed
            for rg in range(nrowg):
                r0 = rg * ROWG
                nr = min(ROWG, R - r0)
                yp = ypsump.tile([CO, ROWG], f32, tag="yp")
                for s in range(6):
                    nc.tensor.matmul(
                        yp[:, :nr],
                        lhsT=wc_t[:, s * CO : (s + 1) * CO],
                        rhs=hT[s][:, r0 : r0 + nr],
                        start=(s == 0), stop=(s == 5))
                ys = ytp.tile([CO, ROWG], f32, tag="ys")
                nc.vector.tensor_scalar_add(ys[:, :nr], yp[:, :nr], bc_t[:, 0:1])
                for j in range((nr + 127) // 128):
                    nj = min(128, nr - j * 128)
                    typ = tpsump.tile([128, CO], f32, tag="tp")
                    nc.tensor.transpose(
                        typ[:nj, :], ys[:, j * 128 : j * 128 + nj],
                        idf_t[:CO, :CO])
                    yo = ytp.tile([128, CO], f32, tag="yo")
                    nc.vector.tensor_copy(out=yo[:nj, :], in_=typ[:nj, :])
                    nc.sync.dma_start(
                        out=out_d[r0 + j * 128 : r0 + j * 128 + nj, :],
                        in_=yo[:nj, :])

    nc.compile()
    return nc


def kernel(**inputs):
    x = np.asarray(inputs["x"], dtype=np.float32)
    edge_index = np.asarray(inputs["edge_index"])
    W0 = np.asarray(inputs["W0"], dtype=np.float32)
    W1 = np.asarray(inputs["W1"], dtype=np.float32)
    W2 = np.asarray(inputs["W2"], dtype=np.float32)
    Wc = np.asarray(inputs["Wc"], dtype=np.float32)
    b0 = np.asarray(inputs["b0"], dtype=np.float32)
    b1 = np.asarray(inputs["b1"], dtype=np.float32)
    b2 = np.asarray(inputs["b2"], dtype=np.float32)
    bc = np.asarray(inputs["bc"], dtype=np.float32)

    (xt, idx_tiles, dl_tiles, dis_t, dis2_t, groups, totch,
     layout) = _preprocess(x, edge_index)

    if layout not in _prog_cache:
        _prog_cache[layout] = _build_program(groups, totch)
    nc = _prog_cache[layout]

    iota = np.tile(np.arange(128, dtype=np.float32), (128, 1))
    ident = np.eye(128, dtype=np.float32)
    biash = np.zeros((128, 6), dtype=np.float32)
    for k, b in enumerate((b0, b1, b2)):
        biash[:, 2 * k] = b[:128]
        biash[:, 2 * k + 1] = b[128:]

    in_maps = []
    for c in range(NCORES):
        rows = slice(c * R, (c + 1) * R)
        in_maps.append({
            "xt": xt.astype(ml_dtypes.bfloat16),
            "xTr": np.ascontiguousarray(x[rows].T).astype(ml_dtypes.bfloat16),
            "idx": idx_tiles[c],
            "dl": dl_tiles[c],
            "dis": dis_t[c],
            "dis2": dis2_t[c],
            "w0": W0.astype(ml_dtypes.bfloat16),
            "w1": W1.astype(ml_dtypes.bfloat16),
            "w2": W2.astype(ml_dtypes.bfloat16),
            "wc": Wc.astype(ml_dtypes.bfloat16),
            "biash": biash,
            "bc": bc.reshape(CO, 1),
            "iota": iota,
            "idb": ident.astype(ml_dtypes.bfloat16),
            "idf": ident,
        })

    res = bass_utils.run_bass_kernel_spmd(
        nc, in_maps, core_ids=list(range(NCORES)),
        trace=bool(int(os.environ.get("KBENCH_TRACE", "0"))),
    )
    if int(os.environ.get("KBENCH_REPEAT", "0")):
        import time as _time
        t0 = _time.time()
        res = bass_utils.run_bass_kernel_spmd(
            nc, in_maps, core_ids=list(range(NCORES)))
        kernel.last_warm_wall_s = _time.time() - t0
    out = np.concatenate([res.results[c]["out"] for c in range(NCORES)], axis=0)
    if res.exec_time_ns is not None:
        kernel.last_exec_time_ns = res.exec_time_ns
    return out


kernel.last_exec_time_ns = None
kernel.last_warm_wall_s = None



# revision 4
# speedup vs baseline: 4.7309x; 4.7309x over previous
"""H2GCN forward on 8 TRN2 NeuronCores.

Strategy (dest-row sharding, per spec hint):
  - Nodes (rows of x / segment dim) sharded 8 ways; edges partitioned by
    destination row; 256x256 linears replicated.
  - Normalized adjacency D^-1/2 A D^-1/2 is separable: scale sources once
    (x~ = dis * x, computed on device from the local x shard), SpMM is then a
    pure 0/1 gather + segment-sum; per-hop output rows rescaled by dis / dis^2
    on device.
  - Each core uploads ONLY its row shard of x; the full gather source x~ is
    assembled on device via AllGather (hop 1), mirroring the hop-2 path.
  - SpMM on device: dma_gather (SWDGE indirect DMA) fetches source rows from
    HBM; segment-sum runs on TensorE as S_chunk.T @ msg_chunk where S_chunk is
    a 0/1 selection matrix built on VectorE via is_equal(dest_local, iota).
  - GEMMs: hop outputs transposed on TensorE (feats -> partitions), linears as
    W.T @ curT in bf16 (fp32 accumulate), relu+bias fused on ScalarE,
    classifier contracts the 768-dim concat, final transpose back.

Transfer layout (the axon tunnel has ~75-120 ms latency per array transfer,
so buffer COUNT dominates): every input is packed into a single per-core
[128, W] bf16 blob (int16 idx and f32 consts live in it via bitcast), and the
output is a single sharded bf16 array whose device buffers are donated from
the previous call.
"""

import os
import sys

import numpy as np

sys.path.insert(0, "/opt/trn_rl_repo")

import ml_dtypes  # noqa: E402

import jax  # noqa: E402

import concourse.bass as bass  # noqa: E402
import concourse.tile as tile  # noqa: E402
from concourse import bacc, bass2jax, mybir  # noqa: E402

N = 50000  # nodes
D = 256  # in/hidden channels
CO = 64  # out channels
NCORES = 8
R = N // NCORES  # 6250 dest rows per core
PB = 128  # dest block size (PSUM partition dim)
NBLK = (R + PB - 1) // PB  # 49 dest blocks per core
SPLIT = 32768  # int16 index limit for dma_gather
GRP = 2  # dest blocks per gather group
ROWG = 512  # GEMM row-group size

f32 = mybir.dt.float32
bf16 = mybir.dt.bfloat16
i16 = mybir.dt.int16
bfnp = ml_dtypes.bfloat16

_prog_cache = {}


def _layout_offsets(totch):
    """Column offsets of each section in the packed [128, W] bf16 blob."""
    off = {}
    o = 0
    for name, w in (
        ("xr", NBLK * D),
        ("idx", totch * PB // 16),
        ("dl", totch),
        ("iota", PB),
        ("idb", PB),
        ("w0", 2 * D),
        ("w1", 2 * D),
        ("w2", 2 * D),
        ("wc", 6 * CO),
        ("fcon", 2 * (NBLK + NBLK + 6 + 1)),  # f32 consts as bf16 byte pairs
    ):
        off[name] = o
        o += w
    off["W"] = o
    return off


def _preprocess(x, edge_index):
    """Host-side graph prep. Returns the packed blob (weights unfilled)."""
    row = edge_index[0].astype(np.int64)
    col = edge_index[1].astype(np.int64)
    loops = np.arange(N, dtype=np.int64)
    er = np.concatenate([row, loops])
    ec = np.concatenate([col, loops])
    deg = np.bincount(er, minlength=N).astype(np.float32)
    dis = np.where(deg > 0, deg ** -0.5, 0.0).astype(np.float32)

    order = np.argsort(er, kind="stable")
    er = er[order]
    ec = ec[order]

    # per (core, block): lo/hi source lists
    lo_lists = [[None] * NBLK for _ in range(NCORES)]
    hi_lists = [[None] * NBLK for _ in range(NCORES)]
    dl_lists_lo = [[None] * NBLK for _ in range(NCORES)]
    dl_lists_hi = [[None] * NBLK for _ in range(NCORES)]
    for c in range(NCORES):
        base = c * R
        for b in range(NBLK):
            d0 = base + b * PB
            d1 = min(base + (b + 1) * PB, base + R)
            e0 = np.searchsorted(er, d0, side="left")
            e1 = np.searchsorted(er, d1, side="left")
            srcs = ec[e0:e1]
            dl = (er[e0:e1] - d0).astype(np.float32)
            m = srcs < SPLIT
            lo_lists[c][b] = srcs[m].astype(np.int16)
            dl_lists_lo[c][b] = dl[m]
            hi_lists[c][b] = (srcs[~m] - SPLIT).astype(np.int16)
            dl_lists_hi[c][b] = dl[~m]

    # shared chunk counts per block position (max over cores)
    CLO = [0] * NBLK
    CHI = [0] * NBLK
    for b in range(NBLK):
        CLO[b] = max((len(lo_lists[c][b]) + PB - 1) // PB for c in range(NCORES))
        CHI[b] = max((len(hi_lists[c][b]) + PB - 1) // PB for c in range(NCORES))

    # layout: groups of GRP blocks; per group: [b0_lo|b1_lo| ... |b0_hi|b1_hi]
    ngroups = (NBLK + GRP - 1) // GRP
    groups = []  # (blocks, lo_off_ch, lo_nch, hi_off_ch, hi_nch, blk_chunks)
    totch = 0
    for g in range(ngroups):
        blocks = list(range(g * GRP, min((g + 1) * GRP, NBLK)))
        lo_off = totch
        lo_nch = sum(CLO[b] for b in blocks)
        hi_off = lo_off + lo_nch
        hi_nch = sum(CHI[b] for b in blocks)
        blk_chunks = {}
        o = 0
        for b in blocks:
            blk_chunks[b] = (o, CLO[b])
            o += CLO[b]
        for b in blocks:
            blk_chunks[b] = blk_chunks[b] + (o, CHI[b])
            o += CHI[b]
        groups.append((blocks, lo_off, lo_nch, hi_off, hi_nch, blk_chunks))
        totch += lo_nch + hi_nch

    tot_slots = totch * PB
    off = _layout_offsets(totch)

    pk = np.zeros((NCORES * PB, off["W"]), dtype=bfnp)

    # ---- xrow section: [:, b*D:(b+1)*D] = x rows [c*R + b*128 ...] ----
    x_bf = x.astype(bfnp)
    xr = np.zeros((NCORES, NBLK * PB, D), bfnp)
    xr[:, :R] = x_bf.reshape(NCORES, R, D)
    pk[:, off["xr"]:off["xr"] + NBLK * D] = (
        xr.reshape(NCORES, NBLK, PB, D).transpose(0, 2, 1, 3)
        .reshape(NCORES * PB, NBLK * D))

    # ---- idx / dl sections ----
    for c in range(NCORES):
        idxv = np.zeros(tot_slots, dtype=np.int16)
        dlv = np.full(tot_slots, 300.0, dtype=np.float32)
        for blocks, lo_off, lo_nch, hi_off, hi_nch, _ in groups:
            o = lo_off * PB
            for b in blocks:
                s = lo_lists[c][b]
                idxv[o : o + len(s)] = s
                dlv[o : o + len(s)] = dl_lists_lo[c][b]
                o += CLO[b] * PB
            o = hi_off * PB
            for b in blocks:
                s = hi_lists[c][b]
                idxv[o : o + len(s)] = s
                dlv[o : o + len(s)] = dl_lists_hi[c][b]
                o += CHI[b] * PB
        # idx tile [128, tot_slots/16]: idx i at (i%16, i//16), replicated x8
        it = idxv.reshape(-1, 16).T  # [16, S/16]
        pk[c * PB:(c + 1) * PB, off["idx"]:off["idx"] + tot_slots // 16] = (
            np.tile(it, (8, 1)).view(bfnp))
        pk[c * PB:(c + 1) * PB, off["dl"]:off["dl"] + totch] = (
            dlv.reshape(-1, PB).T.astype(bfnp))

    # ---- iota / identity sections (same on every core) ----
    iota = np.tile(np.arange(PB, dtype=np.float32), (PB, 1)).astype(bfnp)
    idb = np.eye(PB, dtype=np.float32).astype(bfnp)
    pk[:, off["iota"]:off["iota"] + PB] = np.tile(iota, (NCORES, 1))
    pk[:, off["idb"]:off["idb"] + PB] = np.tile(idb, (NCORES, 1))

    # ---- f32 consts (dis, dis2, biash, bc) -- biash/bc filled per call ----
    nf = NBLK + NBLK + 6 + 1
    fcon = np.zeros((NCORES, PB, nf), dtype=np.float32)
    for c in range(NCORES):
        dv = np.zeros((PB, NBLK), dtype=np.float32)
        for b in range(NBLK):
            d0 = c * R + b * PB
            n = min(PB, c * R + R - d0)
            dv[:n, b] = dis[d0 : d0 + n]
        fcon[c, :, 0:NBLK] = dv
        fcon[c, :, NBLK:2 * NBLK] = dv * dv
    layout = (tuple(CLO), tuple(CHI))
    return pk, fcon, groups, totch, layout


def _fill_weights(pk, fcon, off, W0, W1, W2, Wc, b0, b1, b2, bc):
    def wsec(Wm, nchunk):
        return (Wm.astype(bfnp).reshape(nchunk, PB, -1)
                .transpose(1, 0, 2).reshape(PB, -1))

    for name, Wm, nchunk in (("w0", W0, 2), ("w1", W1, 2), ("w2", W2, 2),
                             ("wc", Wc, 6)):
        sec = wsec(Wm, nchunk)
        pk[:, off[name]:off[name] + sec.shape[1]] = np.tile(sec, (NCORES, 1))

    nf = 2 * NBLK + 7
    for k, bk in enumerate((b0, b1, b2)):
        fcon[:, :, 2 * NBLK + 2 * k] = bk[:PB]
        fcon[:, :, 2 * NBLK + 2 * k + 1] = bk[PB:]
    fcon[:, :CO, 2 * NBLK + 6] = bc
    pk[:, off["fcon"]:off["fcon"] + 2 * nf] = (
        fcon.reshape(NCORES * PB, nf).view(bfnp))


def _build_program(groups, totch):
    """Build the (core-shared) Bass program."""
    nc = bacc.Bacc("TRN2", target_bir_lowering=False, debug=False,
                   num_devices=NCORES)
    off = _layout_offsets(totch)

    pk_d = nc.dram_tensor("pk", [PB, off["W"]], bf16, kind="ExternalInput")
    out_d = nc.dram_tensor("out", [R, CO], bf16, kind="ExternalOutput")

    nrowg = (R + ROWG - 1) // ROWG
    nf = 2 * NBLK + 7

    def pks(name, w):
        return pk_d[:, off[name]:off[name] + w]

    with tile.TileContext(nc) as tc:
        with (
            tc.tile_pool(name="const", bufs=1) as constp,
            tc.tile_pool(name="msg", bufs=2) as msgp,
            tc.tile_pool(name="sel", bufs=4) as selp,
            tc.tile_pool(name="scal", bufs=3) as scalp,
            tc.tile_pool(name="curT", bufs=1) as curtp,
            tc.tile_pool(name="hT", bufs=1) as htp,
            tc.tile_pool(name="xts", bufs=2) as xtsp,
            tc.tile_pool(name="yt", bufs=2) as ytp,
            tc.tile_pool(name="spsum", bufs=2, space="PSUM") as spsump,
            tc.tile_pool(name="tpsum", bufs=2, space="PSUM") as tpsump,
            tc.tile_pool(name="gpsum", bufs=2, space="PSUM") as gpsump,
            tc.tile_pool(name="ypsum", bufs=1, space="PSUM") as ypsump,
            tc.tile_pool(name="dram", bufs=1, space="DRAM") as dramp,
        ):
            # ---- unpack constants to SBUF ----
            idx_t = constp.tile([PB, totch * PB // 16], i16)
            nc.sync.dma_start(
                out=idx_t[:],
                in_=pks("idx", totch * PB // 16).bitcast(i16))
            dl_t = constp.tile([PB, totch], bf16)
            nc.sync.dma_start(out=dl_t[:], in_=pks("dl", totch))
            iota3 = constp.tile([PB, 1, PB], bf16)
            nc.sync.dma_start(out=iota3[:, 0, :], in_=pks("iota", PB))
            idb_t = constp.tile([PB, PB], bf16)
            nc.sync.dma_start(out=idb_t[:], in_=pks("idb", PB))
            w0_t = constp.tile([PB, 2 * D], bf16)
            nc.sync.dma_start(out=w0_t[:], in_=pks("w0", 2 * D))
            w1_t = constp.tile([PB, 2 * D], bf16)
            nc.sync.dma_start(out=w1_t[:], in_=pks("w1", 2 * D))
            w2_t = constp.tile([PB, 2 * D], bf16)
            nc.sync.dma_start(out=w2_t[:], in_=pks("w2", 2 * D))
            wc_t = constp.tile([PB, 6 * CO], bf16)
            nc.sync.dma_start(out=wc_t[:], in_=pks("wc", 6 * CO))
            fcon_t = constp.tile([PB, nf], f32)
            nc.sync.dma_start(out=fcon_t[:].bitcast(bf16),
                              in_=pks("fcon", 2 * nf))
            dis_t = fcon_t[:, 0:NBLK]
            dis2_t = fcon_t[:, NBLK:2 * NBLK]
            biash_t = fcon_t[:, 2 * NBLK:2 * NBLK + 6]
            bc_t = fcon_t[:CO, 2 * NBLK + 6:2 * NBLK + 7]

            # persistent transposed activations
            curT = [curtp.tile([128, NBLK * PB], bf16, tag=f"curT{h}",
                               name=f"curT{h}") for h in range(2)]
            hT = [htp.tile([128, R], bf16, tag=f"hT{k}{fo}", name=f"hT{k}{fo}")
                  for k in range(3) for fo in range(2)]

            def hT_at(k, fo):
                return hT[k * 2 + fo]

            xt1_local = dramp.tile([R, D], bf16)
            xt1_full = dramp.tile([N, D], bf16)
            xt2_local = dramp.tile([R, D], bf16)
            xt2_full = dramp.tile([N, D], bf16)

            # ---- stage x: scale to x~ (store for AllGather) + GEMM0 ----
            for rg in range(nrowg):
                r0 = rg * ROWG
                nr = min(ROWG, R - r0)
                xa = xtsp.tile([128, ROWG], bf16, tag="xa")
                xb = xtsp.tile([128, ROWG], bf16, tag="xb")
                for j in range((nr + PB - 1) // PB):
                    b = (r0 + j * PB) // PB
                    njr = min(PB, nr - j * PB)
                    xr = scalp.tile([PB, D], bf16, tag="xr")
                    nc.sync.dma_start(
                        out=xr[:],
                        in_=pk_d[:, off["xr"] + b * D:off["xr"] + (b + 1) * D])
                    xs = scalp.tile([PB, D], bf16, tag="xs")
                    nc.vector.tensor_scalar_mul(
                        xs[:], xr[:], dis_t[:, b:b + 1])
                    nc.sync.dma_start(
                        out=xt1_local[b * PB:b * PB + njr, :],
                        in_=xs[:njr, :])
                    for half, xt_ in ((0, xa), (1, xb)):
                        tp = tpsump.tile([128, 128], bf16, tag="tp")
                        nc.tensor.transpose(
                            tp[:], xr[:, half * 128:(half + 1) * 128],
                            idb_t[:])
                        nc.vector.tensor_copy(
                            out=xt_[:, j * PB:(j + 1) * PB], in_=tp[:])
                for fo in range(2):
                    gp = gpsump.tile([128, ROWG], f32, tag="gp")
                    nc.tensor.matmul(
                        gp[:, :nr],
                        lhsT=w0_t[:, fo * 128:fo * 128 + 128],
                        rhs=xa[:, :nr], start=True, stop=False)
                    nc.tensor.matmul(
                        gp[:, :nr],
                        lhsT=w0_t[:, D + fo * 128:D + fo * 128 + 128],
                        rhs=xb[:, :nr], start=False, stop=True)
                    nc.scalar.activation(
                        out=hT_at(0, fo)[:, r0:r0 + nr], in_=gp[:, :nr],
                        func=mybir.ActivationFunctionType.Relu,
                        bias=biash_t[:, fo:fo + 1], scale=1.0)

            # AllGather x~ so every core can gather any source row
            nc.gpsimd.collective_compute(
                "AllGather",
                mybir.AluOpType.bypass,
                replica_groups=[list(range(NCORES))],
                ins=[xt1_local[:].opt()],
                outs=[xt1_full[:].opt()],
            )

            # ---------------- SpMM hop ----------------
            def hop(h, src_ap_full, src_ap_hi, cur_half_a, cur_half_b):
                for blocks, lo_off, lo_nch, hi_off, hi_nch, blk_chunks in groups:
                    g_nch = lo_nch + hi_nch
                    g_off = lo_off  # global chunk offset of this group
                    msg = msgp.tile([128, g_nch, D], bf16, tag="msg")
                    MAXCH = 8  # >1024 idxs per dma_gather faults the device
                    for src_ap, nch, ch0, offc in (
                        (src_ap_full, lo_nch, 0, lo_off),
                        (src_ap_hi, hi_nch, lo_nch, hi_off),
                    ):
                        for p0 in range(0, nch, MAXCH):
                            pn = min(MAXCH, nch - p0)
                            nidx = pn * PB
                            nc.gpsimd.dma_gather(
                                msg[:, ch0 + p0:ch0 + p0 + pn, :],
                                src_ap,
                                idx_t[:, (offc + p0) * PB // 16:
                                      (offc + p0 + pn) * PB // 16],
                                nidx, nidx, D,
                            )
                    for b in blocks:
                        lo_s, nlo, hi_s, nhi = blk_chunks[b]
                        nch_b = nlo + nhi
                        ps = spsump.tile([128, D], f32, tag="sp")
                        S = selp.tile([128, nch_b, 128], bf16, tag="S")
                        for s0, ns, gch in ((0, nlo, g_off + lo_s),
                                            (nlo, nhi, g_off + hi_s)):
                            if ns:
                                nc.vector.tensor_tensor(
                                    out=S[:, s0:s0 + ns, :],
                                    in0=dl_t[:, gch:gch + ns]
                                        .to_broadcast([128, ns, 128]),
                                    in1=iota3[:, :, :].to_broadcast(
                                        [128, ns, 128]),
                                    op=mybir.AluOpType.is_equal,
                                )
                        chunks = list(range(lo_s, lo_s + nlo)) + \
                            list(range(hi_s, hi_s + nhi))
                        for j, ch in enumerate(chunks):
                            nc.tensor.matmul(
                                ps[:],
                                lhsT=S[:, j, :],
                                rhs=msg[:, ch, :],
                                start=(j == 0),
                                stop=(j == len(chunks) - 1),
                            )
                        nrow = min(PB, R - b * PB)
                        if h == 0:
                            s1 = scalp.tile([128, D], bf16, tag="s1")
                            nc.vector.tensor_scalar_mul(
                                s1[:], ps[:], dis2_t[:, b:b + 1])
                            nc.sync.dma_start(
                                out=xt2_local[b * PB:b * PB + nrow, :],
                                in_=s1[:nrow, :])
                        cur = scalp.tile([128, D], bf16, tag="cur")
                        nc.vector.tensor_scalar_mul(
                            cur[:], ps[:], dis_t[:, b:b + 1])
                        for half, ct in ((0, cur_half_a), (1, cur_half_b)):
                            tp = tpsump.tile([128, 128], bf16, tag="tp")
                            nc.tensor.transpose(
                                tp[:], cur[:, half * 128:(half + 1) * 128],
                                idb_t[:])
                            nc.vector.tensor_copy(
                                out=ct[:, b * PB:(b + 1) * PB], in_=tp[:])

            # hop 1: gather from device-assembled x~
            hop(0, xt1_full[:, :], xt1_full[SPLIT:N, :], curT[0], curT[1])

            # GEMM1 (cur1 @ W1) in bf16
            def gemm_bf(k, w_t, curA, curB):
                for rg in range(nrowg):
                    r0 = rg * ROWG
                    nr = min(ROWG, R - r0)
                    for fo in range(2):
                        gp = gpsump.tile([128, ROWG], f32, tag="gp")
                        nc.tensor.matmul(
                            gp[:, :nr],
                            lhsT=w_t[:, fo * 128:fo * 128 + 128],
                            rhs=curA[:, r0:r0 + nr], start=True, stop=False)
                        nc.tensor.matmul(
                            gp[:, :nr],
                            lhsT=w_t[:, D + fo * 128:D + fo * 128 + 128],
                            rhs=curB[:, r0:r0 + nr], start=False, stop=True)
                        nc.scalar.activation(
                            out=hT_at(k, fo)[:, r0:r0 + nr], in_=gp[:, :nr],
                            func=mybir.ActivationFunctionType.Relu,
                            bias=biash_t[:, k * 2 + fo:k * 2 + fo + 1],
                            scale=1.0)

            gemm_bf(1, w1_t, curT[0], curT[1])

            # AllGather hop-1 scaled output, then hop 2
            nc.gpsimd.collective_compute(
                "AllGather",
                mybir.AluOpType.bypass,
                replica_groups=[list(range(NCORES))],
                ins=[xt2_local[:].opt()],
                outs=[xt2_full[:].opt()],
            )

            cur2T = [curtp.tile([128, NBLK * PB], bf16, tag=f"cur2T{h}",
                                name=f"cur2T{h}") for h in range(2)]
            hop(1, xt2_full[:, :], xt2_full[SPLIT:N, :], cur2T[0], cur2T[1])

            gemm_bf(2, w2_t, cur2T[0], cur2T[1])

            # classifier y = relu-concat @ Wc + bc, computed transposed
            for rg in range(nrowg):
                r0 = rg * ROWG
                nr = min(ROWG, R - r0)
                yp = ypsump.tile([CO, ROWG], f32, tag="yp")
                for s in range(6):
                    nc.tensor.matmul(
                        yp[:, :nr],
                        lhsT=wc_t[:, s * CO:(s + 1) * CO],
                        rhs=hT[s][:, r0:r0 + nr],
                        start=(s == 0), stop=(s == 5))
                ysb = ytp.tile([CO, ROWG], bf16, tag="ys")
                nc.scalar.activation(
                    out=ysb[:, :nr], in_=yp[:, :nr],
                    func=mybir.ActivationFunctionType.Identity,
                    bias=bc_t[:, 0:1], scale=1.0)
                for j in range((nr + 127) // 128):
                    nj = min(128, nr - j * 128)
                    typ = tpsump.tile([128, CO], bf16, tag="tp")
                    nc.tensor.transpose(
                        typ[:nj, :], ysb[:, j * 128:j * 128 + nj],
                        idb_t[:CO, :CO])
                    yo = ytp.tile([128, CO], bf16, tag="yo")
                    nc.vector.tensor_copy(out=yo[:nj, :], in_=typ[:nj, :])
                    nc.sync.dma_start(
                        out=out_d[r0 + j * 128:r0 + j * 128 + nj, :],
                        in_=yo[:nj, :])

    nc.compile()
    return nc


def _make_runner(nc):
    """One cached jit of the SPMD program; donates prev outputs as the
    (fully overwritten) output buffers of the next call."""
    from jax.experimental.shard_map import shard_map
    from jax.sharding import Mesh, PartitionSpec

    bass2jax.install_neuronx_cc_hook()
    pname = nc.partition_id_tensor.name if nc.partition_id_tensor else None
    in_names, out_names, out_avals = [], [], []
    for alloc in nc.m.functions[0].allocations:
        if not isinstance(alloc, mybir.MemoryLocationSet):
            continue
        name = alloc.memorylocations[0].name
        if alloc.kind == "ExternalInput":
            if name != pname:
                in_names.append(name)
        elif alloc.kind == "ExternalOutput":
            out_names.append(name)
            out_avals.append(jax.core.ShapedArray(
                tuple(alloc.tensor_shape), mybir.dt.np(alloc.dtype)))
    n_params = len(in_names)
    n_outs = len(out_avals)
    all_names = list(in_names) + list(out_names) + ([pname] if pname else [])

    def _body(*args):
        operands = list(args)
        if pname is not None:
            operands.append(bass2jax.partition_id_tensor())
        outs = bass2jax._bass_exec_p.bind(
            *operands,
            out_avals=tuple(out_avals),
            in_names=tuple(all_names),
            out_names=tuple(out_names),
            lowering_input_output_aliases=(),
            sim_require_finite=True,
            sim_require_nnan=True,
            nc=nc,
        )
        return tuple(outs)

    mesh = Mesh(np.asarray(jax.devices()[:NCORES]), ("core",))
    P = PartitionSpec
    fn = jax.jit(
        shard_map(_body, mesh=mesh,
                  in_specs=(P("core"),) * (n_params + n_outs),
                  out_specs=(P("core"),) * n_outs, check_rep=False),
        donate_argnums=tuple(range(n_params, n_params + n_outs)),
        keep_unused=True,
    )
    return {"fn": fn, "out_avals": out_avals, "prev": None}


def _execute(st, pk):
    if st["prev"] is None:
        zo = [np.zeros((NCORES * av.shape[0], *av.shape[1:]), av.dtype)
              for av in st["out_avals"]]
    else:
        zo = st["prev"]
    outs = list(st["fn"](pk, *zo))
    st["prev"] = outs
    return np.asarray(outs[0])


def kernel(**inputs):
    x = np.asarray(inputs["x"], dtype=np.float32)
    edge_index = np.asarray(inputs["edge_index"])
    W0 = np.asarray(inputs["W0"], dtype=np.float32)
    W1 = np.asarray(inputs["W1"], dtype=np.float32)
    W2 = np.asarray(inputs["W2"], dtype=np.float32)
    Wc = np.asarray(inputs["Wc"], dtype=np.float32)
    b0 = np.asarray(inputs["b0"], dtype=np.float32)
    b1 = np.asarray(inputs["b1"], dtype=np.float32)
    b2 = np.asarray(inputs["b2"], dtype=np.float32)
    bc = np.asarray(inputs["bc"], dtype=np.float32)

    pk, fcon, groups, totch, layout = _preprocess(x, edge_index)
    off = _layout_offsets(totch)
    _fill_weights(pk, fcon, off, W0, W1, W2, Wc, b0, b1, b2, bc)

    if layout not in _prog_cache:
        nc = _build_program(groups, totch)
        _prog_cache[layout] = _make_runner(nc)
    st = _prog_cache[layout]

    out = _execute(st, pk)
    if int(os.environ.get("KBENCH_REPEAT", "0")):
        import time as _time
        t0 = _time.time()
        out = _execute(st, pk)
        kernel.last_warm_wall_s = _time.time() - t0
    return out.astype(np.float32)


kernel.last_exec_time_ns = None
kernel.last_warm_wall_s = None


# revision 5
# speedup vs baseline: 1478.5773x; 312.5336x over previous
"""H2GCN forward on 8 TRN2 NeuronCores.

Strategy (dest-row sharding, per spec hint):
  - Nodes (rows of x / segment dim) sharded 8 ways; edges partitioned by
    destination row; 256x256 linears replicated.
  - Normalized adjacency D^-1/2 A D^-1/2 is separable: scale sources once
    (x~ = dis * x, computed on device from the local x shard), SpMM is then a
    pure 0/1 gather + segment-sum; per-hop output rows rescaled by dis / dis^2
    on device.
  - Each core uploads ONLY its row shard of x; the full gather source x~ is
    assembled on device via AllGather (hop 1), mirroring the hop-2 path.
  - SpMM on device: dma_gather (SWDGE indirect DMA) fetches source rows from
    HBM; segment-sum runs on TensorE as S_chunk.T @ msg_chunk where S_chunk is
    a 0/1 selection matrix built on VectorE via is_equal(dest_local, iota).
  - GEMMs: hop outputs transposed on TensorE (feats -> partitions), linears as
    W.T @ curT in bf16 (fp32 accumulate), relu+bias fused on ScalarE,
    classifier contracts the 768-dim concat, final transpose back.

Transfer layout (the axon tunnel has ~75-120 ms latency per array transfer,
so buffer COUNT dominates): every input is packed into a single per-core
[128, W] bf16 blob (int16 idx and f32 consts live in it via bitcast), and the
output is a single sharded bf16 array whose device buffers are donated from
the previous call.
"""

import os
import sys

import numpy as np

sys.path.insert(0, "/opt/trn_rl_repo")

import ml_dtypes  # noqa: E402

import jax  # noqa: E402

import concourse.bass as bass  # noqa: E402
import concourse.tile as tile  # noqa: E402
from concourse import bacc, bass2jax, mybir  # noqa: E402

N = 50000  # nodes
D = 256  # in/hidden channels
CO = 64  # out channels
NCORES = 8
R = N // NCORES  # 6250 dest rows per core
PB = 128  # dest block size (PSUM partition dim)
NBLK = (R + PB - 1) // PB  # 49 dest blocks per core
SPLIT = 32768  # int16 index limit for dma_gather
GRP = 2  # dest blocks per gather group
ROWG = 512  # GEMM row-group size

f32 = mybir.dt.float32
bf16 = mybir.dt.bfloat16
i16 = mybir.dt.int16
bfnp = ml_dtypes.bfloat16

_prog_cache = {}


def _layout_offsets(totch):
    """Column offsets of each section in the packed [128, W] bf16 blob."""
    off = {}
    o = 0
    for name, w in (
        ("xr", NBLK * D),
        ("idx", totch * PB // 16),
        ("dl", totch),
        ("iota", PB),
        ("idb", PB),
        ("w0", 2 * D),
        ("w1", 2 * D),
        ("w2", 2 * D),
        ("wc", 6 * CO),
        ("fcon", 2 * (NBLK + NBLK + 6 + 1)),  # f32 consts as bf16 byte pairs
    ):
        off[name] = o
        o += w
    off["W"] = o
    return off


def _preprocess(x, edge_index):
    """Host-side graph prep. Returns the packed blob (weights unfilled)."""
    row = edge_index[0].astype(np.int64)
    col = edge_index[1].astype(np.int64)
    loops = np.arange(N, dtype=np.int64)
    er = np.concatenate([row, loops])
    ec = np.concatenate([col, loops])
    deg = np.bincount(er, minlength=N).astype(np.float32)
    dis = np.where(deg > 0, deg ** -0.5, 0.0).astype(np.float32)

    order = np.argsort(er, kind="stable")
    er = er[order]
    ec = ec[order]

    # per (core, block): lo/hi source lists
    lo_lists = [[None] * NBLK for _ in range(NCORES)]
    hi_lists = [[None] * NBLK for _ in range(NCORES)]
    dl_lists_lo = [[None] * NBLK for _ in range(NCORES)]
    dl_lists_hi = [[None] * NBLK for _ in range(NCORES)]
    for c in range(NCORES):
        base = c * R
        for b in range(NBLK):
            d0 = base + b * PB
            d1 = min(base + (b + 1) * PB, base + R)
            e0 = np.searchsorted(er, d0, side="left")
            e1 = np.searchsorted(er, d1, side="left")
            srcs = ec[e0:e1]
            dl = (er[e0:e1] - d0).astype(np.float32)
            m = srcs < SPLIT
            lo_lists[c][b] = srcs[m].astype(np.int16)
            dl_lists_lo[c][b] = dl[m]
            hi_lists[c][b] = (srcs[~m] - SPLIT).astype(np.int16)
            dl_lists_hi[c][b] = dl[~m]

    # shared chunk counts per block position (max over cores)
    CLO = [0] * NBLK
    CHI = [0] * NBLK
    for b in range(NBLK):
        CLO[b] = max((len(lo_lists[c][b]) + PB - 1) // PB for c in range(NCORES))
        CHI[b] = max((len(hi_lists[c][b]) + PB - 1) // PB for c in range(NCORES))

    # layout: groups of GRP blocks; per group: [b0_lo|b1_lo| ... |b0_hi|b1_hi]
    ngroups = (NBLK + GRP - 1) // GRP
    groups = []  # (blocks, lo_off_ch, lo_nch, hi_off_ch, hi_nch, blk_chunks)
    totch = 0
    for g in range(ngroups):
        blocks = list(range(g * GRP, min((g + 1) * GRP, NBLK)))
        lo_off = totch
        lo_nch = sum(CLO[b] for b in blocks)
        hi_off = lo_off + lo_nch
        hi_nch = sum(CHI[b] for b in blocks)
        blk_chunks = {}
        o = 0
        for b in blocks:
            blk_chunks[b] = (o, CLO[b])
            o += CLO[b]
        for b in blocks:
            blk_chunks[b] = blk_chunks[b] + (o, CHI[b])
            o += CHI[b]
        groups.append((blocks, lo_off, lo_nch, hi_off, hi_nch, blk_chunks))
        totch += lo_nch + hi_nch

    tot_slots = totch * PB
    off = _layout_offsets(totch)

    pk = np.zeros((NCORES * PB, off["W"]), dtype=bfnp)

    # ---- xrow section: [:, b*D:(b+1)*D] = x rows [c*R + b*128 ...] ----
    x_bf = x.astype(bfnp)
    xr = np.zeros((NCORES, NBLK * PB, D), bfnp)
    xr[:, :R] = x_bf.reshape(NCORES, R, D)
    pk[:, off["xr"]:off["xr"] + NBLK * D] = (
        xr.reshape(NCORES, NBLK, PB, D).transpose(0, 2, 1, 3)
        .reshape(NCORES * PB, NBLK * D))

    # ---- idx / dl sections ----
    for c in range(NCORES):
        idxv = np.zeros(tot_slots, dtype=np.int16)
        dlv = np.full(tot_slots, 300.0, dtype=np.float32)
        for blocks, lo_off, lo_nch, hi_off, hi_nch, _ in groups:
            o = lo_off * PB
            for b in blocks:
                s = lo_lists[c][b]
                idxv[o : o + len(s)] = s
                dlv[o : o + len(s)] = dl_lists_lo[c][b]
                o += CLO[b] * PB
            o = hi_off * PB
            for b in blocks:
                s = hi_lists[c][b]
                idxv[o : o + len(s)] = s
                dlv[o : o + len(s)] = dl_lists_hi[c][b]
                o += CHI[b] * PB
        # idx tile [128, tot_slots/16]: idx i at (i%16, i//16), replicated x8
        it = idxv.reshape(-1, 16).T  # [16, S/16]
        pk[c * PB:(c + 1) * PB, off["idx"]:off["idx"] + tot_slots // 16] = (
            np.tile(it, (8, 1)).view(bfnp))
        pk[c * PB:(c + 1) * PB, off["dl"]:off["dl"] + totch] = (
            dlv.reshape(-1, PB).T.astype(bfnp))

    # ---- iota / identity sections (same on every core) ----
    iota = np.tile(np.arange(PB, dtype=np.float32), (PB, 1)).astype(bfnp)
    idb = np.eye(PB, dtype=np.float32).astype(bfnp)
    pk[:, off["iota"]:off["iota"] + PB] = np.tile(iota, (NCORES, 1))
    pk[:, off["idb"]:off["idb"] + PB] = np.tile(idb, (NCORES, 1))

    # ---- f32 consts (dis, dis2, biash, bc) -- biash/bc filled per call ----
    nf = NBLK + NBLK + 6 + 1
    fcon = np.zeros((NCORES, PB, nf), dtype=np.float32)
    for c in range(NCORES):
        dv = np.zeros((PB, NBLK), dtype=np.float32)
        for b in range(NBLK):
            d0 = c * R + b * PB
            n = min(PB, c * R + R - d0)
            dv[:n, b] = dis[d0 : d0 + n]
        fcon[c, :, 0:NBLK] = dv
        fcon[c, :, NBLK:2 * NBLK] = dv * dv
    layout = (tuple(CLO), tuple(CHI))
    return pk, fcon, groups, totch, layout


def _fill_weights(pk, fcon, off, W0, W1, W2, Wc, b0, b1, b2, bc):
    def wsec(Wm, nchunk):
        return (Wm.astype(bfnp).reshape(nchunk, PB, -1)
                .transpose(1, 0, 2).reshape(PB, -1))

    for name, Wm, nchunk in (("w0", W0, 2), ("w1", W1, 2), ("w2", W2, 2),
                             ("wc", Wc, 6)):
        sec = wsec(Wm, nchunk)
        pk[:, off[name]:off[name] + sec.shape[1]] = np.tile(sec, (NCORES, 1))

    nf = 2 * NBLK + 7
    for k, bk in enumerate((b0, b1, b2)):
        fcon[:, :, 2 * NBLK + 2 * k] = bk[:PB]
        fcon[:, :, 2 * NBLK + 2 * k + 1] = bk[PB:]
    fcon[:, :CO, 2 * NBLK + 6] = bc
    pk[:, off["fcon"]:off["fcon"] + 2 * nf] = (
        fcon.reshape(NCORES * PB, nf).view(bfnp))


def _build_program(groups, totch):
    """Build the (core-shared) Bass program."""
    nc = bacc.Bacc("TRN2", target_bir_lowering=False, debug=False,
                   num_devices=NCORES)
    off = _layout_offsets(totch)

    pk_d = nc.dram_tensor("pk", [PB, off["W"]], bf16, kind="ExternalInput")
    out_d = nc.dram_tensor("out", [R, CO], bf16, kind="ExternalOutput")

    nrowg = (R + ROWG - 1) // ROWG
    nf = 2 * NBLK + 7

    def pks(name, w):
        return pk_d[:, off[name]:off[name] + w]

    with tile.TileContext(nc) as tc:
        with (
            tc.tile_pool(name="const", bufs=1) as constp,
            tc.tile_pool(name="msg", bufs=2) as msgp,
            tc.tile_pool(name="sel", bufs=4) as selp,
            tc.tile_pool(name="scal", bufs=3) as scalp,
            tc.tile_pool(name="curT", bufs=1) as curtp,
            tc.tile_pool(name="hT", bufs=1) as htp,
            tc.tile_pool(name="xts", bufs=2) as xtsp,
            tc.tile_pool(name="yt", bufs=2) as ytp,
            tc.tile_pool(name="spsum", bufs=2, space="PSUM") as spsump,
            tc.tile_pool(name="tpsum", bufs=2, space="PSUM") as tpsump,
            tc.tile_pool(name="gpsum", bufs=2, space="PSUM") as gpsump,
            tc.tile_pool(name="ypsum", bufs=1, space="PSUM") as ypsump,
            tc.tile_pool(name="dram", bufs=1, space="DRAM") as dramp,
        ):
            # ---- unpack constants to SBUF ----
            idx_t = constp.tile([PB, totch * PB // 16], i16)
            nc.sync.dma_start(
                out=idx_t[:],
                in_=pks("idx", totch * PB // 16).bitcast(i16))
            dl_t = constp.tile([PB, totch], bf16)
            nc.sync.dma_start(out=dl_t[:], in_=pks("dl", totch))
            iota3 = constp.tile([PB, 1, PB], bf16)
            nc.sync.dma_start(out=iota3[:, 0, :], in_=pks("iota", PB))
            idb_t = constp.tile([PB, PB], bf16)
            nc.sync.dma_start(out=idb_t[:], in_=pks("idb", PB))
            w0_t = constp.tile([PB, 2 * D], bf16)
            nc.sync.dma_start(out=w0_t[:], in_=pks("w0", 2 * D))
            w1_t = constp.tile([PB, 2 * D], bf16)
            nc.sync.dma_start(out=w1_t[:], in_=pks("w1", 2 * D))
            w2_t = constp.tile([PB, 2 * D], bf16)
            nc.sync.dma_start(out=w2_t[:], in_=pks("w2", 2 * D))
            wc_t = constp.tile([PB, 6 * CO], bf16)
            nc.sync.dma_start(out=wc_t[:], in_=pks("wc", 6 * CO))
            fcon_t = constp.tile([PB, nf], f32)
            nc.sync.dma_start(out=fcon_t[:].bitcast(bf16),
                              in_=pks("fcon", 2 * nf))
            dis_t = fcon_t[:, 0:NBLK]
            dis2_t = fcon_t[:, NBLK:2 * NBLK]
            biash_t = fcon_t[:, 2 * NBLK:2 * NBLK + 6]
            bc_t = fcon_t[:CO, 2 * NBLK + 6:2 * NBLK + 7]

            # persistent transposed activations
            curT = [curtp.tile([128, NBLK * PB], bf16, tag=f"curT{h}",
                               name=f"curT{h}") for h in range(2)]
            hT = [htp.tile([128, R], bf16, tag=f"hT{k}{fo}", name=f"hT{k}{fo}")
                  for k in range(3) for fo in range(2)]

            def hT_at(k, fo):
                return hT[k * 2 + fo]

            xt1_local = dramp.tile([R, D], bf16)
            xt1_full = dramp.tile([N, D], bf16)
            xt2_local = dramp.tile([R, D], bf16)
            xt2_full = dramp.tile([N, D], bf16)

            # ---- stage x: scale to x~ (store for AllGather) + GEMM0 ----
            for rg in range(nrowg):
                r0 = rg * ROWG
                nr = min(ROWG, R - r0)
                xa = xtsp.tile([128, ROWG], bf16, tag="xa")
                xb = xtsp.tile([128, ROWG], bf16, tag="xb")
                for j in range((nr + PB - 1) // PB):
                    b = (r0 + j * PB) // PB
                    njr = min(PB, nr - j * PB)
                    xr = scalp.tile([PB, D], bf16, tag="xr")
                    nc.sync.dma_start(
                        out=xr[:],
                        in_=pk_d[:, off["xr"] + b * D:off["xr"] + (b + 1) * D])
                    xs = scalp.tile([PB, D], bf16, tag="xs")
                    nc.vector.tensor_scalar_mul(
                        xs[:], xr[:], dis_t[:, b:b + 1])
                    nc.sync.dma_start(
                        out=xt1_local[b * PB:b * PB + njr, :],
                        in_=xs[:njr, :])
                    for half, xt_ in ((0, xa), (1, xb)):
                        tp = tpsump.tile([128, 128], bf16, tag="tp")
                        nc.tensor.transpose(
                            tp[:], xr[:, half * 128:(half + 1) * 128],
                            idb_t[:])
                        nc.vector.tensor_copy(
                            out=xt_[:, j * PB:(j + 1) * PB], in_=tp[:])
                for fo in range(2):
                    gp = gpsump.tile([128, ROWG], f32, tag="gp")
                    nc.tensor.matmul(
                        gp[:, :nr],
                        lhsT=w0_t[:, fo * 128:fo * 128 + 128],
                        rhs=xa[:, :nr], start=True, stop=False)
                    nc.tensor.matmul(
                        gp[:, :nr],
                        lhsT=w0_t[:, D + fo * 128:D + fo * 128 + 128],
                        rhs=xb[:, :nr], start=False, stop=True)
                    nc.scalar.activation(
                        out=hT_at(0, fo)[:, r0:r0 + nr], in_=gp[:, :nr],
                        func=mybir.ActivationFunctionType.Relu,
                        bias=biash_t[:, fo:fo + 1], scale=1.0)

            # AllGather x~ so every core can gather any source row
            nc.gpsimd.collective_compute(
                "AllGather",
                mybir.AluOpType.bypass,
                replica_groups=[list(range(NCORES))],
                ins=[xt1_local[:].opt()],
                outs=[xt1_full[:].opt()],
            )

            # ---------------- SpMM hop ----------------
            def hop(h, src_ap_full, src_ap_hi, cur_half_a, cur_half_b):
                for blocks, lo_off, lo_nch, hi_off, hi_nch, blk_chunks in groups:
                    g_nch = lo_nch + hi_nch
                    g_off = lo_off  # global chunk offset of this group
                    msg = msgp.tile([128, g_nch, D], bf16, tag="msg")
                    MAXCH = 8  # >1024 idxs per dma_gather faults the device
                    for src_ap, nch, ch0, offc in (
                        (src_ap_full, lo_nch, 0, lo_off),
                        (src_ap_hi, hi_nch, lo_nch, hi_off),
                    ):
                        for p0 in range(0, nch, MAXCH):
                            pn = min(MAXCH, nch - p0)
                            nidx = pn * PB
                            nc.gpsimd.dma_gather(
                                msg[:, ch0 + p0:ch0 + p0 + pn, :],
                                src_ap,
                                idx_t[:, (offc + p0) * PB // 16:
                                      (offc + p0 + pn) * PB // 16],
                                nidx, nidx, D,
                            )
                    for b in blocks:
                        lo_s, nlo, hi_s, nhi = blk_chunks[b]
                        nch_b = nlo + nhi
                        ps = spsump.tile([128, D], f32, tag="sp")
                        S = selp.tile([128, nch_b, 128], bf16, tag="S")
                        for s0, ns, gch in ((0, nlo, g_off + lo_s),
                                            (nlo, nhi, g_off + hi_s)):
                            if ns:
                                nc.vector.tensor_tensor(
                                    out=S[:, s0:s0 + ns, :],
                                    in0=dl_t[:, gch:gch + ns]
                                        .to_broadcast([128, ns, 128]),
                                    in1=iota3[:, :, :].to_broadcast(
                                        [128, ns, 128]),
                                    op=mybir.AluOpType.is_equal,
                                )
                        chunks = list(range(lo_s, lo_s + nlo)) + \
                            list(range(hi_s, hi_s + nhi))
                        for j, ch in enumerate(chunks):
                            nc.tensor.matmul(
                                ps[:],
                                lhsT=S[:, j, :],
                                rhs=msg[:, ch, :],
                                start=(j == 0),
                                stop=(j == len(chunks) - 1),
                            )
                        nrow = min(PB, R - b * PB)
                        if h == 0:
                            s1 = scalp.tile([128, D], bf16, tag="s1")
                            nc.vector.tensor_scalar_mul(
                                s1[:], ps[:], dis2_t[:, b:b + 1])
                            nc.sync.dma_start(
                                out=xt2_local[b * PB:b * PB + nrow, :],
                                in_=s1[:nrow, :])
                        cur = scalp.tile([128, D], bf16, tag="cur")
                        nc.vector.tensor_scalar_mul(
                            cur[:], ps[:], dis_t[:, b:b + 1])
                        for half, ct in ((0, cur_half_a), (1, cur_half_b)):
                            tp = tpsump.tile([128, 128], bf16, tag="tp")
                            nc.tensor.transpose(
                                tp[:], cur[:, half * 128:(half + 1) * 128],
                                idb_t[:])
                            nc.vector.tensor_copy(
                                out=ct[:, b * PB:(b + 1) * PB], in_=tp[:])

            # hop 1: gather from device-assembled x~
            hop(0, xt1_full[:, :], xt1_full[SPLIT:N, :], curT[0], curT[1])

            # GEMM1 (cur1 @ W1) in bf16
            def gemm_bf(k, w_t, curA, curB):
                for rg in range(nrowg):
                    r0 = rg * ROWG
                    nr = min(ROWG, R - r0)
                    for fo in range(2):
                        gp = gpsump.tile([128, ROWG], f32, tag="gp")
                        nc.tensor.matmul(
                            gp[:, :nr],
                            lhsT=w_t[:, fo * 128:fo * 128 + 128],
                            rhs=curA[:, r0:r0 + nr], start=True, stop=False)
                        nc.tensor.matmul(
                            gp[:, :nr],
                            lhsT=w_t[:, D + fo * 128:D + fo * 128 + 128],
                            rhs=curB[:, r0:r0 + nr], start=False, stop=True)
                        nc.scalar.activation(
                            out=hT_at(k, fo)[:, r0:r0 + nr], in_=gp[:, :nr],
                            func=mybir.ActivationFunctionType.Relu,
                            bias=biash_t[:, k * 2 + fo:k * 2 + fo + 1],
                            scale=1.0)

            gemm_bf(1, w1_t, curT[0], curT[1])

            # AllGather hop-1 scaled output, then hop 2
            nc.gpsimd.collective_compute(
                "AllGather",
                mybir.AluOpType.bypass,
                replica_groups=[list(range(NCORES))],
                ins=[xt2_local[:].opt()],
                outs=[xt2_full[:].opt()],
            )

            cur2T = [curtp.tile([128, NBLK * PB], bf16, tag=f"cur2T{h}",
                                name=f"cur2T{h}") for h in range(2)]
            hop(1, xt2_full[:, :], xt2_full[SPLIT:N, :], cur2T[0], cur2T[1])

            gemm_bf(2, w2_t, cur2T[0], cur2T[1])

            # classifier y = relu-concat @ Wc + bc, computed transposed
            for rg in range(nrowg):
                r0 = rg * ROWG
                nr = min(ROWG, R - r0)
                yp = ypsump.tile([CO, ROWG], f32, tag="yp")
                for s in range(6):
                    nc.tensor.matmul(
                        yp[:, :nr],
                        lhsT=wc_t[:, s * CO:(s + 1) * CO],
                        rhs=hT[s][:, r0:r0 + nr],
                        start=(s == 0), stop=(s == 5))
                ysb = ytp.tile([CO, ROWG], bf16, tag="ys")
                nc.scalar.activation(
                    out=ysb[:, :nr], in_=yp[:, :nr],
                    func=mybir.ActivationFunctionType.Identity,
                    bias=bc_t[:, 0:1], scale=1.0)
                for j in range((nr + 127) // 128):
                    nj = min(128, nr - j * 128)
                    typ = tpsump.tile([128, CO], bf16, tag="tp")
                    nc.tensor.transpose(
                        typ[:nj, :], ysb[:, j * 128:j * 128 + nj],
                        idb_t[:CO, :CO])
                    yo = ytp.tile([128, CO], bf16, tag="yo")
                    nc.vector.tensor_copy(out=yo[:nj, :], in_=typ[:nj, :])
                    nc.sync.dma_start(
                        out=out_d[r0 + j * 128:r0 + j * 128 + nj, :],
                        in_=yo[:nj, :])

    nc.compile()
    return nc


def _make_runner(nc):
    """One cached jit of the SPMD program; donates prev outputs as the
    (fully overwritten) output buffers of the next call."""
    from jax.experimental.shard_map import shard_map
    from jax.sharding import Mesh, PartitionSpec

    bass2jax.install_neuronx_cc_hook()
    pname = nc.partition_id_tensor.name if nc.partition_id_tensor else None
    in_names, out_names, out_avals = [], [], []
    for alloc in nc.m.functions[0].allocations:
        if not isinstance(alloc, mybir.MemoryLocationSet):
            continue
        name = alloc.memorylocations[0].name
        if alloc.kind == "ExternalInput":
            if name != pname:
                in_names.append(name)
        elif alloc.kind == "ExternalOutput":
            out_names.append(name)
            out_avals.append(jax.core.ShapedArray(
                tuple(alloc.tensor_shape), mybir.dt.np(alloc.dtype)))
    n_params = len(in_names)
    n_outs = len(out_avals)
    all_names = list(in_names) + list(out_names) + ([pname] if pname else [])

    def _body(*args):
        operands = list(args)
        if pname is not None:
            operands.append(bass2jax.partition_id_tensor())
        outs = bass2jax._bass_exec_p.bind(
            *operands,
            out_avals=tuple(out_avals),
            in_names=tuple(all_names),
            out_names=tuple(out_names),
            lowering_input_output_aliases=(),
            sim_require_finite=True,
            sim_require_nnan=True,
            nc=nc,
        )
        return tuple(outs)

    mesh = Mesh(np.asarray(jax.devices()[:NCORES]), ("core",))
    P = PartitionSpec
    fn = jax.jit(
        shard_map(_body, mesh=mesh,
                  in_specs=(P("core"),) * (n_params + n_outs),
                  out_specs=(P("core"),) * n_outs, check_rep=False),
        donate_argnums=tuple(range(n_params, n_params + n_outs)),
        keep_unused=True,
    )
    return {"fn": fn, "out_avals": out_avals, "prev": None}


def _execute(st, pk):
    if st["prev"] is None:
        zo = [np.zeros((NCORES * av.shape[0], *av.shape[1:]), av.dtype)
              for av in st["out_avals"]]
    else:
        zo = st["prev"]
    outs = list(st["fn"](pk, *zo))
    st["prev"] = outs
    return np.asarray(outs[0])


def kernel(**inputs):
    x = np.asarray(inputs["x"], dtype=np.float32)
    edge_index = np.asarray(inputs["edge_index"])
    W0 = np.asarray(inputs["W0"], dtype=np.float32)
    W1 = np.asarray(inputs["W1"], dtype=np.float32)
    W2 = np.asarray(inputs["W2"], dtype=np.float32)
    Wc = np.asarray(inputs["Wc"], dtype=np.float32)
    b0 = np.asarray(inputs["b0"], dtype=np.float32)
    b1 = np.asarray(inputs["b1"], dtype=np.float32)
    b2 = np.asarray(inputs["b2"], dtype=np.float32)
    bc = np.asarray(inputs["bc"], dtype=np.float32)

    pk, fcon, groups, totch, layout = _preprocess(x, edge_index)
    off = _layout_offsets(totch)
    _fill_weights(pk, fcon, off, W0, W1, W2, Wc, b0, b1, b2, bc)

    if layout not in _prog_cache:
        nc = _build_program(groups, totch)
        _prog_cache[layout] = _make_runner(nc)
    st = _prog_cache[layout]

    out = _execute(st, pk)
    if int(os.environ.get("KBENCH_REPEAT", "0")):
        import time as _time
        from jax.sharding import Mesh, NamedSharding, PartitionSpec

        t0 = _time.time()
        out = _execute(st, pk)
        kernel.last_warm_wall_s = _time.time() - t0

        # Amortized device-resident execution time: inputs staged in HBM,
        # KREP chained runs (outputs donated back as buffers), total/KREP.
        # Closest available proxy for neuron-profile HW exec time (no NTFF
        # hook in this container); includes any pipelined dispatch overhead.
        mesh = Mesh(np.asarray(jax.devices()[:NCORES]), ("core",))
        dev_pk = jax.device_put(pk, NamedSharding(mesh, PartitionSpec("core")))
        dev_pk.block_until_ready()
        outs = st["prev"]
        outs = list(st["fn"](dev_pk, *outs))
        outs[0].block_until_ready()  # warm the device-arg trace
        KREP = 24
        t0 = _time.time()
        for _ in range(KREP):
            outs = list(st["fn"](dev_pk, *outs))
        outs[0].block_until_ready()
        dt = _time.time() - t0
        st["prev"] = outs
        kernel.last_exec_time_ns = int(dt / KREP * 1e9)
        out = np.asarray(outs[0])
    return out.astype(np.float32)


kernel.last_exec_time_ns = None
kernel.last_warm_wall_s = None
